# revision 35
# baseline (speedup 1.0000x reference)
"""AttentionBlock (GroupNorm + 1x1-conv QKV self-attention + residual) on 8 TRN2 cores.

Sharding: data-parallel over batch B=4 x sequence-parallel over the 4096
tokens (2 cores per batch element, each handling 2048 query rows; K/V and
GroupNorm are computed redundantly per core pair). The host rotates x's
token columns per core so each core's query rows are always columns
0..2047 — the kernel stays SPMD and needs no per-core offsets, and the
residual comes from the bf16 x tile (no separate fp32 x shipment).

Per-core device kernel (attention matmuls in bf16, GN stats fp32):
  - x is striped over 4 HWDGE queues; weights ride the gpsimd (SWDGE)
    queue so bulk x bandwidth is unshared. GroupNorm stats overlap the
    input DMA (bn_stats per arriving piece); the group combine is two
    small PE matmuls (one gathers both channel-tiles' mean/E[x^2], one
    broadcasts mean/rstd back per channel) + one Rsqrt.
  - GroupNorm is folded into the QKV weights: h = scale_c*x + shift_c, so
    q/k/v come straight from x with per-channel-scaled weights + effective
    biases collapsed into per-partition vectors.
  - q/k are 2x-replicated across partition strips via col-packed
    projection matmuls, enabling 2x row-packed S^T matmuls (K=32).
    The first two S^T groups are issued before the V projection so the
    scalar engine starts exp'ing while the PE computes v^T.
  - Softmax denominator: 4-way col-packed ones-matmuls (positions
    0/32/64/96) accumulate column sums for two exp-groups per issue;
    non-final chunks mask-combine + gpsimd partition_all_reduce; the
    final chunk uses two tiny PE matmuls (select + broadcast) so the
    tail never waits on the 3.5us gpsimd reduce.
  - P*V accumulates over m-blocks into a 2-bank PSUM tile as out_att[e,n];
    each chunk's epilogue is software-pipelined into the next chunk's
    stream; the final epilogue is sliced into 256-col pieces.
"""
import sys

sys.path.insert(0, "/opt/trn_rl_repo")

import ml_dtypes
import numpy as np

import concourse.bass as bass
import concourse.bass_isa as bass_isa
import concourse.tile as tile
from concourse.tile_rust import add_dep_helper
from concourse import bacc, mybir
from concourse.bass_utils import run_bass_kernel_spmd

F32 = mybir.dt.float32
BF16 = mybir.dt.bfloat16
FP8 = mybir.dt.float8e4
USE_FP8 = True          # fp8 P/V + DoubleRow P*V matmuls
PDT = FP8 if USE_FP8 else BF16

B, C, H, W = 4, 256, 64, 64
N = H * W          # 4096 tokens
NQ = N // 2        # 2048 query rows per core
D = C // 8         # 32 qk dim
G = 32             # groups
GS = C // G        # 8 channels per group
EPS = 1e-5
P = 128            # partitions
CT = C // P        # 2 channel tiles
GT = G // CT       # 16 groups per channel tile
CH = 512           # nq chunk
NCH = NQ // CH     # 4 chunks
MB = 128           # m block
NMB = N // MB      # 32 m blocks
NG2 = NMB // 2     # 16 groups of 2 m-blocks per chunk
XP = 512           # x DMA piece size
SM_SCALE = float(D) ** -0.5

_CACHE = {}
_last_in_maps = None


def _build():
    if "nc" in _CACHE:
        return _CACHE["nc"]

    nc = bacc.Bacc("TRN2", target_bir_lowering=False, debug=False, num_devices=8)

    x_ext = nc.declare_dram_parameter("x", [C, N], BF16, isOutput=False)
    wqt_ext = nc.declare_dram_parameter("wqt", [C, D], F32, isOutput=False)
    wkt_ext = nc.declare_dram_parameter("wkt", [C, D], F32, isOutput=False)
    wvt_ext = nc.declare_dram_parameter("wvt", [C, C], F32, isOutput=False)
    wpt_ext = nc.declare_dram_parameter("wpt", [C, C], F32, isOutput=False)
    bq_ext = nc.declare_dram_parameter("bq", [D, 1], F32, isOutput=False)
    bk_ext = nc.declare_dram_parameter("bk", [D, 1], F32, isOutput=False)
    bv_ext = nc.declare_dram_parameter("bv", [C, 1], F32, isOutput=False)
    bp_ext = nc.declare_dram_parameter("bp", [C, 1], F32, isOutput=False)
    gamma_ext = nc.declare_dram_parameter("gamma", [C, 1], F32, isOutput=False)
    beta_ext = nc.declare_dram_parameter("beta", [C, 1], F32, isOutput=False)
    ind16_ext = nc.declare_dram_parameter("ind16", [P, GT], F32, isOutput=False)
    indb_ext = nc.declare_dram_parameter("indb", [GT, P], F32, isOutput=False)
    out_ext = nc.declare_dram_parameter("out", [C, NQ], F32, isOutput=True)

    NP = N // XP  # 4 x pieces per channel tile

    with tile.TileContext(nc) as tc:
        with tc.tile_pool(name="const", bufs=1) as const, \
             tc.tile_pool(name="small", bufs=1) as small:
            # ---- x DMA first, striped over 4 HWDGE queues ----
            x_r = [const.tile([P, N], BF16, tag=f"xr{t}", name=f"xr{t}") for t in range(CT)]
            qengs = [nc.sync, nc.scalar, nc.gpsimd]
            for t in range(CT):
                cs = slice(t * P, (t + 1) * P)
                for pc in range(NP):
                    ps_ = slice(pc * XP, (pc + 1) * XP)
                    qengs[(t * NP + pc) % 3].dma_start(out=x_r[t][:, ps_], in_=x_ext[cs, ps_])

            # ---- critical weights on HWDGE right behind x; rest on SWDGE ----
            wqt_sb, wkt_sb, wvt_sb, wpt_sb = [], [], [], []
            gamma_sb, beta_sb, bv_sb, bp_sb = [], [], [], []
            for t in range(CT):
                cs = slice(t * P, (t + 1) * P)
                w3 = const.tile([P, C], F32, tag=f"wvt{t}", name=f"wvt{t}")
                qengs[t % 2].dma_start(out=w3, in_=wvt_ext[cs, :])
                wvt_sb.append(w3)
                w1 = const.tile([P, D], F32, tag=f"wqt{t}", name=f"wqt{t}")
                nc.sync.dma_start(out=w1, in_=wqt_ext[cs, :])
                wqt_sb.append(w1)
                w2 = const.tile([P, D], F32, tag=f"wkt{t}", name=f"wkt{t}")
                nc.scalar.dma_start(out=w2, in_=wkt_ext[cs, :])
                wkt_sb.append(w2)
            ind16_sb = small.tile([P, GT], F32, tag="ind16")
            nc.gpsimd.dma_start(out=ind16_sb, in_=ind16_ext[:])
            indb_sb = small.tile([GT, P], F32, tag="indb")
            nc.gpsimd.dma_start(out=indb_sb, in_=indb_ext[:])
            for t in range(CT):
                cs = slice(t * P, (t + 1) * P)
                for lst, ext, nm in (
                    (gamma_sb, gamma_ext, "gam"),
                    (beta_sb, beta_ext, "bet"),
                    (bv_sb, bv_ext, "bv"),
                    (bp_sb, bp_ext, "bp"),
                ):
                    tl = small.tile([P, 1], F32, tag=f"{nm}{t}", name=f"{nm}{t}")
                    nc.gpsimd.dma_start(out=tl, in_=ext[cs, :])
                    lst.append(tl)
            bq_sb = small.tile([D, 1], F32, tag="bq")
            nc.gpsimd.dma_start(out=bq_sb, in_=bq_ext[:])
            bk_sb = small.tile([D, 1], F32, tag="bk")
            nc.gpsimd.dma_start(out=bk_sb, in_=bk_ext[:])
            for t in range(CT):
                cs = slice(t * P, (t + 1) * P)
                w4 = const.tile([P, C], F32, tag=f"wpt{t}", name=f"wpt{t}")
                nc.gpsimd.dma_start(out=w4, in_=wpt_ext[cs, :])
                wpt_sb.append(w4)

            # ---- small constants (vector memsets) ----
            ones32 = small.tile([P, 32], PDT, tag="ones32")
            nc.vector.memset(ones32, 1.0)
            ones1 = small.tile([1, P], F32, tag="ones1")
            nc.vector.memset(ones1, 1.0)
            inv32 = small.tile([P, 1], F32, tag="inv32")
            nc.vector.memset(inv32, 1.0 / 32.0)
            warmw = small.tile([P, P], BF16, tag="warmw")
            nc.vector.memset(warmw, 0.001)
            warmx = small.tile([P, CH], BF16, tag="warmx")
            nc.vector.memset(warmx, 0.001)
            eps_sb = small.tile([GT, 1], F32, tag="eps")
            nc.vector.memset(eps_sb, EPS)
            # preload the sqrt activation table during the DMA wait
            scr16 = small.tile([GT, 1], F32, tag="scr16")
            nc.scalar.activation(
                out=scr16, in_=eps_sb,
                func=mybir.ActivationFunctionType.Sqrt,
            )

            # ---- persistent tiles ----
            wqt_h = [const.tile([P, D], BF16, tag=f"wqth{t}", name=f"wqth{t}") for t in range(CT)]
            wkt_h = [const.tile([P, D], BF16, tag=f"wkth{t}", name=f"wkth{t}") for t in range(CT)]
            wvt_h = [const.tile([P, C], BF16, tag=f"wvth{t}", name=f"wvth{t}") for t in range(CT)]
            wpt_h = [const.tile([P, C], BF16, tag=f"wpth{t}", name=f"wpth{t}") for t in range(CT)]
            xqb = [const.tile([P, NQ], F32, tag=f"xqb{t}", name=f"xqb{t}") for t in range(CT)]
            scale_sb = [small.tile([P, 1], F32, tag=f"scale{t}", name=f"scale{t}") for t in range(CT)]
            shift_sb = [small.tile([P, 1], F32, tag=f"shift{t}", name=f"shift{t}") for t in range(CT)]
            q_rep = const.tile([64, NQ], BF16, tag="qrep")
            k_rep = const.tile([64, N], BF16, tag="krep")
            vt_h = const.tile([P, NMB, C], PDT, tag="vth")

            # ---- GroupNorm stats overlapped with the x DMA ----
            with tc.tile_pool(name="gn", bufs=2) as gn, \
                 tc.tile_pool(name="gnps", bufs=1, space="PSUM") as gnps:
                # HAM warm-up: dummy matmuls during the DMA wait so the PE
                # clock is at 8/8 when the real projections start. A burst
                # up front, then two per arriving x piece to stay warm.
                warm_ps = gnps.tile([P, CH], F32, tag="warm")
                for i in range(14):
                    nc.tensor.matmul(warm_ps, warmw, warmx, start=True, stop=True)

                stats = [gn.tile([P, NP, nc.vector.BN_STATS_DIM], F32,
                                 tag=f"st{t}", name=f"st{t}") for t in range(CT)]
                for t in range(CT):
                    for pc in range(NP):
                        st_inst = nc.vector.bn_stats(
                            out=stats[t][:, pc, :],
                            in_=x_r[t][:, pc * XP:(pc + 1) * XP],
                        )
                        wm = nc.tensor.matmul(warm_ps, warmw, warmx,
                                              start=True, stop=True)
                        add_dep_helper(wm.ins, st_inst.ins, sync=True,
                                       reason="pace warmup with x arrival")
                        nc.tensor.matmul(warm_ps, warmw, warmx, start=True, stop=True)
                # merged combine for both channel tiles:
                # m4 cols = [mean_t0, mean_t1, E2_t0, E2_t1]
                m4 = gn.tile([P, 4], F32, tag="m4")
                for t in range(CT):
                    mv = gn.tile([P, nc.vector.BN_AGGR_DIM], F32, tag=f"mv{t}", name=f"mv{t}")
                    nc.vector.bn_aggr(out=mv, in_=stats[t])
                    nc.vector.tensor_copy(out=m4[:, t:t + 1], in_=mv[:, 0:1])
                    msq = gn.tile([P, 1], F32, tag=f"msq{t}", name=f"msq{t}")
                    nc.vector.tensor_mul(out=msq, in0=mv[:, 0:1], in1=mv[:, 0:1])
                    nc.vector.tensor_add(out=m4[:, 2 + t:3 + t], in0=mv[:, 1:2], in1=msq)

                gnp = gnps.tile([P, 8], F32, tag="gnp")
                gps4 = gnp[0:GT, 0:4]
                nc.tensor.matmul(gps4, ind16_sb, m4, start=True, stop=True,
                                 skip_group_check=True)
                gsb4 = gn.tile([GT, 4], F32, tag="gsb4")
                nc.vector.tensor_copy(out=gsb4, in_=gps4)
                mg2 = gn.tile([GT, 2], F32, tag="mg2")
                nc.vector.tensor_mul(out=mg2, in0=gsb4[:, 0:2], in1=gsb4[:, 0:2])
                varg = gn.tile([GT, 2], F32, tag="varg")
                nc.vector.tensor_sub(out=varg, in0=gsb4[:, 2:4], in1=mg2)
                sd = gn.tile([GT, 2], F32, tag="sd")
                nc.scalar.activation(
                    out=sd, in_=varg,
                    func=mybir.ActivationFunctionType.Sqrt,
                    bias=eps_sb, scale=1.0,
                )
                # g4 cols = [mean_t0, mean_t1, rstd_t0, rstd_t1]
                g4 = gn.tile([GT, 4], F32, tag="g4")
                nc.vector.tensor_copy(out=g4[:, 0:2], in_=gsb4[:, 0:2])
                nc.vector.reciprocal(out=g4[:, 2:4], in_=sd)
                bc = gnp[:, 4:8]
                nc.tensor.matmul(bc, indb_sb, g4, start=True, stop=True,
                                 skip_group_check=True)
                for t in range(CT):
                    nc.vector.tensor_mul(out=scale_sb[t], in0=gamma_sb[t], in1=bc[:, 2 + t:3 + t])
                    sh1 = gn.tile([P, 1], F32, tag=f"sh1{t}", name=f"sh1{t}")
                    nc.vector.tensor_mul(out=sh1, in0=bc[:, t:t + 1], in1=scale_sb[t])
                    nc.vector.tensor_sub(out=shift_sb[t], in0=beta_sb[t], in1=sh1)

                # ---- fold GN scale into the projection weights ----
                for t in range(CT):
                    nc.vector.tensor_scalar_mul(out=wqt_h[t], in0=wqt_sb[t], scalar1=scale_sb[t])
                    nc.vector.tensor_scalar_mul(out=wkt_h[t], in0=wkt_sb[t], scalar1=scale_sb[t])
                    nc.vector.tensor_scalar_mul(out=wvt_h[t], in0=wvt_sb[t], scalar1=scale_sb[t])
                for t in range(CT):
                    nc.vector.tensor_copy(out=wpt_h[t], in_=wpt_sb[t])

                # ---- effective biases are computed AFTER the q matmuls
                # start (PE FIFO: the bias chain's cross-engine ping-pong
                # must not sit in front of the projections) ----

            with tc.tile_pool(name="bps", bufs=1, space="PSUM") as bps, \
                 tc.tile_pool(name="qkps", bufs=1, space="PSUM") as qkps:
                bpt = bps.tile([P, 8], F32, tag="bpt")
                warm2 = bps.tile([P, CH], F32, tag="warm2")
                # keep the PE clock warm across the GN-chain micro-gaps
                for i in range(6):
                    nc.tensor.matmul(warm2, warmw, warmx, start=True, stop=True)

                def q_mms(ch2):
                    qp = qkps.tile([64, 2 * CH], F32, tag="qkp", bufs=2, name="qp")
                    for half in range(2):
                        ns = slice((2 * ch2 + half) * CH, (2 * ch2 + half + 1) * CH)
                        hs = slice(half * CH, (half + 1) * CH)
                        for t in range(CT):
                            for j in range(2):
                                nc.tensor.matmul(
                                    qp[32 * j:32 * (j + 1), hs], wqt_h[t], x_r[t][:, ns],
                                    start=(t == 0), stop=(t == CT - 1),
                                    tile_position=(0, 32 * j),
                                    skip_group_check=True,
                                )
                    return qp

                def q_evac(ch2, qp):
                    ns2a = slice(2 * ch2 * CH, (2 * ch2 + 1) * CH)
                    ns2b = slice((2 * ch2 + 1) * CH, 2 * (ch2 + 1) * CH)
                    nc.scalar.activation(
                        out=q_rep[:, ns2a], in_=qp[:, 0:CH],
                        func=mybir.ActivationFunctionType.Identity,
                        bias=bq_rep[0:64, :], scale=1.0,
                    )
                    nc.vector.tensor_scalar_add(
                        out=q_rep[:, ns2b], in0=qp[:, CH:2 * CH], scalar1=bq_rep[0:64, :],
                    )

                bq_eff = small.tile([D, 1], F32, tag="bqe")
                bk_eff = small.tile([D, 1], F32, tag="bke")
                bq_rep = small.tile([64, 1], F32, tag="bqrep")
                bk_rep = small.tile([64, 1], F32, tag="bkrep")

                qp0 = q_mms(0)
                # q/k biases: 4 tiny matmuls + DVE adds, ready before the
                # first evacuation drains the double-buffered qp
                psq = bpt[0:D, 0:1]
                psk = bpt[0:D, 1:2]
                for t in range(CT):
                    nc.tensor.matmul(psq, wqt_sb[t], shift_sb[t], start=(t == 0), stop=(t == CT - 1),
                                     skip_group_check=True)
                    nc.tensor.matmul(psk, wkt_sb[t], shift_sb[t], start=(t == 0), stop=(t == CT - 1),
                                     skip_group_check=True)
                nc.vector.tensor_add(out=bq_eff, in0=psq, in1=bq_sb)
                nc.vector.tensor_add(out=bk_eff, in0=psk, in1=bk_sb)
                for j in range(2):
                    nc.vector.tensor_copy(out=bq_rep[32 * j:32 * (j + 1), :], in_=bq_eff)
                    nc.vector.tensor_copy(out=bk_rep[32 * j:32 * (j + 1), :], in_=bk_eff)
                q_evac(0, qp0)
                qp1 = q_mms(1)
                q_evac(1, qp1)

                for ch2 in range(N // (2 * CH)):
                    kp = qkps.tile([64, 2 * CH], F32, tag="qkp", bufs=2, name="kp")
                    for half in range(2):
                        ns = slice((2 * ch2 + half) * CH, (2 * ch2 + half + 1) * CH)
                        hs = slice(half * CH, (half + 1) * CH)
                        for t in range(CT):
                            for j in range(2):
                                nc.tensor.matmul(
                                    kp[32 * j:32 * (j + 1), hs], wkt_h[t], x_r[t][:, ns],
                                    start=(t == 0), stop=(t == CT - 1),
                                    tile_position=(0, 32 * j),
                                    skip_group_check=True,
                                )
                    ns2a = slice(2 * ch2 * CH, (2 * ch2 + 1) * CH)
                    ns2b = slice((2 * ch2 + 1) * CH, 2 * (ch2 + 1) * CH)
                    nc.scalar.activation(
                        out=k_rep[:, ns2a], in_=kp[:, 0:CH],
                        func=mybir.ActivationFunctionType.Identity,
                        bias=bk_rep[0:64, :], scale=1.0,
                    )
                    nc.vector.tensor_scalar_add(
                        out=k_rep[:, ns2b], in0=kp[:, CH:2 * CH], scalar1=bk_rep[0:64, :],
                    )

                # v/p biases + residual base (needed only by the epilogues)
                bv_eff = [small.tile([P, 1], F32, tag=f"bve{e}", name=f"bve{e}") for e in range(CT)]
                for e in range(CT):
                    ps3 = bpt[:, 2 + e:3 + e]
                    for t in range(CT):
                        nc.tensor.matmul(
                            ps3, wvt_sb[t][:, e * P:(e + 1) * P], shift_sb[t],
                            start=(t == 0), stop=(t == CT - 1),
                            skip_group_check=True,
                        )
                    nc.vector.tensor_add(out=bv_eff[e], in0=ps3, in1=bv_sb[e])
                for f in range(CT):
                    ps4 = bpt[:, 4 + f:5 + f]
                    for e in range(CT):
                        nc.tensor.matmul(
                            ps4, wpt_sb[e][:, f * P:(f + 1) * P], bv_eff[e],
                            start=(e == 0), stop=(e == CT - 1),
                            skip_group_check=True,
                        )
                    bp_eff = small.tile([P, 1], F32, tag=f"bpe{f}", name=f"bpe{f}")
                    nc.vector.tensor_add(out=bp_eff, in0=ps4, in1=bp_sb[f])
                    # residual base: bf16 x (this core's query half) + bp_eff
                    nc.vector.tensor_scalar_add(out=xqb[f], in0=x_r[f][:, 0:NQ], scalar1=bp_eff)

            # ---- attention ----
            with tc.tile_pool(name="stps", bufs=2, space="PSUM") as stps, \
                 tc.tile_pool(name="attps", bufs=1, space="PSUM") as attps, \
                 tc.tile_pool(name="pp", bufs=6) as pp, \
                 tc.tile_pool(name="attsb", bufs=4) as attsb, \
                 tc.tile_pool(name="osb", bufs=4) as osb, \
                 tc.tile_pool(name="rsb", bufs=2) as rsb:

                def emit_st(ns, g):
                    """S^T for m-block pair g: 2 row-packed matmuls + exp."""
                    stg = stps.tile([P, 2, CH], F32, tag="stg")
                    for j in range(2):
                        mb = g * 2 + j
                        nc.tensor.matmul(
                            stg[:, j, :],
                            k_rep[32 * j:32 * (j + 1), mb * MB:(mb + 1) * MB],
                            q_rep[32 * j:32 * (j + 1), ns],
                            start=True, stop=True,
                            tile_position=(32 * j, 0),
                            skip_group_check=True,
                        )
                    pg = pp.tile([P, 2, CH], PDT, tag="pg")
                    nc.scalar.activation(
                        out=pg, in_=stg,
                        func=mybir.ActivationFunctionType.Exp,
                        scale=SM_SCALE,
                    )
                    return pg

                # pre-issue the first two S^T groups of chunk 0 so exp
                # starts while the PE computes v^T
                pre_pg = [emit_st(slice(0, CH), g) for g in range(2)]

                # ---- v^T (2-m-block granules, evacs alternate scalar/DVE) ----
                with tc.tile_pool(name="vtps", bufs=2, space="PSUM") as vtps:
                    for vg in range(NMB // 2):
                        vp = vtps.tile([P, 2, C], F32, tag="vp")
                        for mloc in range(2):
                            mb = vg * 2 + mloc
                            ms = slice(mb * MB, (mb + 1) * MB)
                            for t in range(CT):
                                nc.tensor.matmul(
                                    vp[:, mloc, :], x_r[t][:, ms], wvt_h[t],
                                    start=(t == 0), stop=(t == CT - 1),
                                )
                        nc.scalar.activation(
                            out=vt_h[:, vg * 2, :], in_=vp[:, 0, :],
                            func=mybir.ActivationFunctionType.Copy,
                        )
                        nc.vector.tensor_copy(out=vt_h[:, vg * 2 + 1, :], in_=vp[:, 1, :])

                with tc.tile_pool(name="rsps", bufs=1, space="PSUM") as rsps:
                    pend_a = None

                    def emit_epilogue(ep, final=False):
                        ns_p, att2_p, rs_p = ep
                        rec_bc = rsb.tile([P, CH], F32, tag="recbc")
                        att_sb2 = attsb.tile([P, CT * CH], BF16, tag="attsb2")
                        if final:
                            # att cast on scalar; denominator via two tiny
                            # PE matmuls (select partitions {0,32,64,96},
                            # then broadcast) instead of the gpsimd reduce
                            nc.scalar.activation(
                                out=att_sb2, in_=att2_p,
                                func=mybir.ActivationFunctionType.Copy,
                            )
                            rs_sb = rsb.tile([P, CH], F32, tag="rssb")
                            nc.vector.tensor_copy(out=rs_sb, in_=rs_p)
                            # (final cast stays whole on scalar: DVE is busy
                            # with the denominator evacuations here)
                            sel_ps = stps.tile([P, 2 * CH], F32, tag="stg")
                            nc.tensor.matmul(
                                sel_ps[0:1, 0:CH], inv32, rs_sb,
                                start=True, stop=True,
                            )
                            r1_sb = rsb.tile([1, CH], F32, tag="r1sb")
                            nc.vector.tensor_copy(out=r1_sb, in_=sel_ps[0:1, 0:CH])
                            bc_ps = stps.tile([P, 2, CH], F32, tag="stg")
                            nc.tensor.matmul(
                                bc_ps[:, 0, :], ones1, r1_sb,
                                start=True, stop=True,
                            )
                            for sl in range(2):
                                ss = slice(sl * (CH // 2), (sl + 1) * (CH // 2))
                                nc.vector.reciprocal(out=rec_bc[:, ss], in_=bc_ps[:, 0, ss])
                        else:
                            nc.scalar.activation(
                                out=att_sb2[:, 0:CH], in_=att2_p[:, 0:CH],
                                func=mybir.ActivationFunctionType.Copy,
                            )
                            nc.vector.tensor_copy(
                                out=att_sb2[:, CH:2 * CH], in_=att2_p[:, CH:2 * CH],
                            )
                            rs_sb = rsb.tile([P, CH], F32, tag="rssb")
                            nc.vector.tensor_scalar_mul(out=rs_sb, in0=rs_p, scalar1=1.0 / 32.0)
                            rsum = rsb.tile([P, CH], F32, tag="rsum")
                            nc.gpsimd.partition_all_reduce(
                                rsum, rs_sb, channels=P, reduce_op=bass_isa.ReduceOp.add,
                            )
                        pjs = []
                        pjc_inst = None
                        for f in range(CT):
                            pj = rsps.tile([P, CH], F32, tag="pj", name=f"pj{f}")
                            for e in range(CT):
                                nc.tensor.matmul(
                                    pj, wpt_h[e][:, f * P:(f + 1) * P],
                                    att_sb2[:, e * CH:(e + 1) * CH],
                                    start=(e == 0), stop=(e == CT - 1),
                                )
                            pjc = osb.tile([P, CH], F32, tag="pjc", name=f"pjc{f}")
                            if final:
                                pjc_inst = nc.scalar.activation(
                                    out=pjc, in_=pj,
                                    func=mybir.ActivationFunctionType.Copy,
                                )
                            else:
                                pjc_inst = nc.vector.tensor_copy(out=pjc, in_=pj)
                            pjs.append(pjc)
                        if not final:
                            rec_inst = nc.vector.reciprocal(out=rec_bc, in_=rsum)
                            # keep the reciprocal behind the pj copies in the
                            # DVE stream so it never blocks the proj path
                            add_dep_helper(rec_inst.ins, pjc_inst.ins, sync=False,
                                           reason="recip after pj copies")
                        nsl = 2 if final else 1
                        sw = CH // nsl
                        for sl in range(nsl):
                            for f in range(CT):
                                fs = slice(f * P, (f + 1) * P)
                                ss = slice(sl * sw, (sl + 1) * sw)
                                gs_ = slice(ns_p.start + sl * sw, ns_p.start + (sl + 1) * sw)
                                t1 = osb.tile([P, sw], F32, tag=f"t1{sl}", name=f"t1{sl}")
                                nc.vector.tensor_mul(out=t1, in0=pjs[f][:, ss], in1=rec_bc[:, ss])
                                o = osb.tile([P, sw], F32, tag=f"o{sl}", name=f"o{sl}")
                                nc.vector.tensor_add(out=o, in0=t1, in1=xqb[f][:, gs_])
                                (nc.sync if f == 0 else nc.scalar).dma_start(
                                    out=out_ext[fs, gs_], in_=o)

                    # p_tiles[(ch, g)] — S^T runs two groups ahead of PV,
                    # crossing chunk boundaries so the exp stream never
                    # stalls at a boundary
                    p_tiles = {(0, 0): pre_pg[0], (0, 1): pre_pg[1]}
                    for ch in range(NCH):
                        ns = slice(ch * CH, (ch + 1) * CH)
                        rs = rsps.tile([P, CH], F32, tag="rs")
                        att2 = None
                        for g in range(NG2):
                            la = g + 2
                            if la < NG2:
                                p_tiles[(ch, la)] = emit_st(ns, la)
                            elif ch + 1 < NCH:
                                nsn = slice((ch + 1) * CH, (ch + 2) * CH)
                                p_tiles[(ch + 1, la - NG2)] = emit_st(nsn, la - NG2)
                            if g == 0:
                                if pend_a is not None:
                                    emit_epilogue(pend_a)
                                    pend_a = None
                                # att2 allocated after the previous chunk's
                                # cast is issued so the buffer-reuse dep holds
                                att2 = attps.tile([P, CT * CH], F32, tag="att2")
                            if g % 2 == 1:
                                # 4-way col-packed denominator matmuls for
                                # groups g-1, g; ones32 writes each strip sum
                                # to 32 partitions (scaled by 1/32 later)
                                for idx in range(4):
                                    pgx = p_tiles[(ch, g - 1 + idx // 2)]
                                    nc.tensor.matmul(
                                        rs[32 * idx:32 * (idx + 1), :],
                                        ones32, pgx[:, idx % 2, :],
                                        start=(g == 1), stop=(g == NG2 - 1),
                                        tile_position=(0, 32 * idx),
                                        skip_group_check=True,
                                    )
                            # P*V: with fp8, one DoubleRow matmul per
                            # e-strip contracts both m-blocks of the group
                            pg = p_tiles[(ch, g)]
                            for e in range(CT):
                                if USE_FP8:
                                    nc.tensor.matmul(
                                        att2[:, e * CH:(e + 1) * CH],
                                        vt_h[:, 2 * g:2 * g + 2, e * P:(e + 1) * P],
                                        pg,
                                        start=(g == 0), stop=(g == NG2 - 1),
                                        perf_mode=mybir.MatmulPerfMode.DoubleRow,
                                        skip_group_check=True,
                                    )
                                else:
                                    for j in range(2):
                                        mb = 2 * g + j
                                        nc.tensor.matmul(
                                            att2[:, e * CH:(e + 1) * CH],
                                            vt_h[:, mb, e * P:(e + 1) * P],
                                            pg[:, j, :],
                                            start=(mb == 0), stop=(mb == NMB - 1),
                                            skip_group_check=True,
                                        )
                        pend_a = (ns, att2, rs)
                    emit_epilogue(pend_a, final=True)

    nc.compile()
    _CACHE["nc"] = nc
    return nc


def make_in_maps(inputs):
    return _make_in_maps(**inputs)


def _make_in_maps(x, gamma, beta, wq, bq, wk, bk, wv, bv, wp, bp):
    x = np.ascontiguousarray(np.asarray(x, dtype=np.float32))

    ind16 = np.zeros((P, GT), np.float32)
    for c in range(P):
        ind16[c, c // GS] = 1.0 / GS
    indb = np.zeros((GT, P), np.float32)
    for c in range(P):
        indb[c // GS, c] = 1.0

    common = {
        "wqt": np.ascontiguousarray(np.asarray(wq, np.float32).T),
        "wkt": np.ascontiguousarray(np.asarray(wk, np.float32).T),
        "wvt": np.ascontiguousarray(np.asarray(wv, np.float32).T),
        "wpt": np.ascontiguousarray(np.asarray(wp, np.float32).T),
        "bq": np.asarray(bq, np.float32).reshape(D, 1),
        "bk": np.asarray(bk, np.float32).reshape(D, 1),
        "bv": np.asarray(bv, np.float32).reshape(C, 1),
        "bp": np.asarray(bp, np.float32).reshape(C, 1),
        "gamma": np.asarray(gamma, np.float32).reshape(C, 1),
        "beta": np.asarray(beta, np.float32).reshape(C, 1),
        "ind16": ind16,
        "indb": indb,
    }

    xf = x.reshape(B, C, N)
    xh = np.ascontiguousarray(xf.astype(ml_dtypes.bfloat16))
    in_maps = []
    for core in range(8):
        b, half = core // 2, core % 2
        m = dict(common)
        if half == 0:
            m["x"] = xh[b]
        else:
            # rotate tokens so this core's query rows are columns 0..NQ-1
            m["x"] = np.ascontiguousarray(
                np.concatenate([xh[b][:, NQ:], xh[b][:, :NQ]], axis=1)
            )
        in_maps.append(m)
    return in_maps


def kernel(x, gamma, beta, wq, bq, wk, bk, wv, bv, wp, bp):
    nc = _build()
    in_maps = _make_in_maps(x, gamma, beta, wq, bq, wk, bk, wv, bv, wp, bp)

    global _last_in_maps
    _last_in_maps = in_maps
    res = run_bass_kernel_spmd(nc, in_maps, list(range(8)))

    y = np.empty((B, C, N), np.float32)
    for core in range(8):
        b, half = core // 2, core % 2
        y[b][:, half * NQ:(half + 1) * NQ] = res.results[core]["out"]
    return y.reshape(B, C, H, W)
